# revision 9
# baseline (speedup 1.0000x reference)
"""DegreeQuantileConverter Trainium2 kernel.

deg (B,S,1) f32 -> out (B,S,12) f32 = log(w + 1e-30) where w are the
piecewise-linear interpolation weights of deg onto the quantile grid
q = [0,1,2,4,...,1024], with rows where deg >= 1024 forced to w = 1.

Math: with c_j = clip((d - q_j)/(q_{j+1}-q_j), 0, 1) for j=0..10 the
weights telescope:  w_0 = 1-c_0, w_j = c_{j-1}-c_j, w_11 = c_10.
Since q_j/(q_{j+1}-q_j) == 1 for j>=1, z_j = d*inv_j - 1 (inv_j a power
of two), which keeps every value bit-identical to the reference's
(d-lo)/(hi-lo) path.  The deg>=1024 all-ones override is applied on the
host (cheap boolean mask on the gathered result).

Sharding: batch 128 -> 16 rows per core x 8 cores, each core sees its
shard as [128 partitions x 2048 cols]; output is written channel-major
[128, 12, 2048] per core and re-interleaved on the host.
"""

import numpy as np

import concourse.bacc as bacc
import concourse.mybir as mybir
import concourse.tile as tile
from concourse.bass_utils import run_bass_kernel_spmd

AF = mybir.ActivationFunctionType
OP = mybir.AluOpType
F32 = mybir.dt.float32

B, S, K = 128, 16384, 12
NCORES = 8
P = 128
ELEMS = (B // NCORES) * S      # 262144 per core
COLS = ELEMS // P              # 2048
F = 1024                       # free-dim tile size
NT = COLS // F                 # 2 tiles per core

QL = [0.0, 1.0, 2.0, 4.0, 8.0, 16.0, 32.0, 64.0, 128.0, 256.0, 512.0, 1024.0]
INV = [1.0] + [1.0 / (QL[j + 1] - QL[j]) for j in range(1, 11)]

# The device Ln table is only accurate for inputs in ~[1e-19, 1e19], but we
# need ln(w + 1e-30) with w in {0} u [3e-8, 1].  So compute
# Ln(w * 2^50 + 1e-30 * 2^50) on device (inputs then span [1.1e-15, 1.1e15])
# and subtract 50*ln2 on the host.
LN_SCALE = float(np.float32(2.0**50))
LN_BIAS = float(np.float32(np.float64(np.float32(1e-30)) * 2.0**50))
LN_OFFSET = np.float32(50.0 * np.log(np.float64(2.0)))

# channel j's clip runs on DVE for j <= DVE_CLIP_MAX, else on GPSIMD
DVE_CLIP_MAX = 4


def build_program():
    nc = bacc.Bacc("TRN2", target_bir_lowering=False, debug=False, num_devices=NCORES)
    # register the ln-bias constant so activation(bias=LN_BIAS) can resolve it
    eps_t = nc.alloc_sbuf_tensor("const-float32-lnbias", [128, 1], F32)
    nc.gpsimd.memset(eps_t.ap(), LN_BIAS)
    nc.const_aps.aps[(F32, LN_BIAS)] = eps_t.ap()
    nc.all_engine_barrier()
    d_ext = nc.declare_dram_parameter("degrees", [P, COLS], F32, isOutput=False)
    out_ext = nc.declare_dram_parameter("out", [P, K, COLS], F32, isOutput=True)

    with tile.TileContext(nc) as tc:
        with (
            tc.tile_pool(name="dp", bufs=2) as dp,
            tc.tile_pool(name="cp", bufs=2) as cp,
            tc.tile_pool(name="sp", bufs=2) as sp,
        ):
            for t in range(NT):
                d = dp.tile([P, F], F32, tag="d")
                nc.sync.dma_start(out=d[:], in_=d_ext[:, t * F : (t + 1) * F])

                stg = sp.tile([P, K * F], F32, tag="stg")

                c = []
                for j in range(11):
                    cj = cp.tile([P, F], F32, tag=f"c{j}")
                    if j == 0:
                        # c_0 = clip(d, 0, 1)
                        nc.vector.tensor_scalar(
                            cj[:], d[:], 0.0, 1.0, OP.max, OP.min
                        )
                    else:
                        # z_j = d*inv_j - 1, then clip in place
                        nc.gpsimd.tensor_scalar(
                            cj[:], d[:], INV[j], 1.0, OP.mult, OP.subtract
                        )
                        eng = nc.vector if j <= DVE_CLIP_MAX else nc.gpsimd
                        eng.tensor_scalar(cj[:], cj[:], 0.0, 1.0, OP.max, OP.min)
                    c.append(cj)

                # w_0 = 1 - c_0
                nc.vector.tensor_scalar(
                    stg[:, 0:F], c[0][:], -1.0, 1.0, OP.mult, OP.add
                )
                # w_j = c_{j-1} - c_j
                for j in range(1, 11):
                    nc.vector.tensor_tensor(
                        stg[:, j * F : (j + 1) * F], c[j - 1][:], c[j][:], OP.subtract
                    )

                # out = ln(w + eps); channel 11 reads c_10 directly.
                # Ln emitted in 3-channel groups so output DMA can overlap.
                for g in range(3):
                    j0, j1 = 3 * g, 3 * (g + 1)
                    nc.scalar.activation(
                        stg[:, j0 * F : j1 * F],
                        stg[:, j0 * F : j1 * F],
                        AF.Ln,
                        bias=LN_BIAS,
                        scale=LN_SCALE,
                    )
                nc.scalar.activation(
                    stg[:, 9 * F : 11 * F],
                    stg[:, 9 * F : 11 * F],
                    AF.Ln,
                    bias=LN_BIAS,
                    scale=LN_SCALE,
                )
                nc.scalar.activation(
                    stg[:, 11 * F : 12 * F], c[10][:], AF.Ln, bias=LN_BIAS, scale=LN_SCALE
                )

                for g in range(4):
                    j0, j1 = 3 * g, 3 * (g + 1)
                    nc.sync.dma_start(
                        out=out_ext[:, j0:j1, t * F : (t + 1) * F],
                        in_=stg[:, j0 * F : j1 * F].rearrange(
                            "p (j f) -> p j f", j=3
                        ),
                    )
    nc.compile()
    return nc


_CACHE = {}
RUN_KWARGS = {}  # test harness can set e.g. {"trace": True} for profiling


def kernel(degrees, quantile_values):
    q = np.asarray(quantile_values, dtype=np.float32)
    assert np.array_equal(q, np.array(QL, dtype=np.float32)), "unexpected quantile grid"

    deg = np.ascontiguousarray(np.asarray(degrees, dtype=np.float32)[..., 0])  # (B,S)
    shards = deg.reshape(NCORES, P, COLS)

    if "nc" not in _CACHE:
        _CACHE["nc"] = build_program()
    nc = _CACHE["nc"]

    in_maps = [{"degrees": np.ascontiguousarray(shards[i])} for i in range(NCORES)]
    res = run_bass_kernel_spmd(nc, in_maps, list(range(NCORES)), **RUN_KWARGS)
    _CACHE["last_result"] = res
    outs = np.stack([res.results[i]["out"] for i in range(NCORES)])  # (8,128,12,2048)

    full = (
        outs.transpose(0, 1, 3, 2)  # (8,128,2048,12) — element order, channel last
        .reshape(B, S, K)
        .astype(np.float32, copy=True)
    )
    full -= LN_OFFSET
    full[deg >= np.float32(1024.0)] = np.float32(0.0)
    return full


# revision 12
# speedup vs baseline: 7.0555x; 7.0555x over previous
"""DegreeQuantileConverter Trainium2 kernel.

deg (B,S,1) f32 -> out (B,S,12) f32 = log(w + 1e-30) where w are the
piecewise-linear interpolation weights of deg onto the quantile grid
q = [0,1,2,4,...,1024], with rows where deg >= 1024 forced to w = 1.

Math: with c_j = clip((d - q_j)/(q_{j+1}-q_j), 0, 1) for j=0..10 the
weights telescope:  w_0 = 1-c_0, w_j = c_{j-1}-c_j, w_11 = c_10.
Since q_j/(q_{j+1}-q_j) == 1 for j>=1, z_j = d*inv_j - 1 (inv_j a power
of two), which keeps every value bit-identical to the reference's
(d-lo)/(hi-lo) path.  The deg>=1024 all-ones override is applied on the
host (cheap boolean mask on the gathered result).

Sharding: batch 128 -> 16 rows per core x 8 cores, each core sees its
shard as [128 partitions x 2048 cols]; output is written channel-major
[128, 12, 2048] per core and re-interleaved on the host.
"""

import numpy as np

import concourse.bacc as bacc
import concourse.mybir as mybir
import concourse.tile as tile
from concourse.bass_utils import run_bass_kernel_spmd

AF = mybir.ActivationFunctionType
OP = mybir.AluOpType
F32 = mybir.dt.float32

B, S, K = 128, 16384, 12
NCORES = 8
P = 128
ELEMS = (B // NCORES) * S      # 262144 per core
COLS = ELEMS // P              # 2048
F = 1024                       # free-dim tile size
NT = COLS // F                 # 2 tiles per core

QL = [0.0, 1.0, 2.0, 4.0, 8.0, 16.0, 32.0, 64.0, 128.0, 256.0, 512.0, 1024.0]
INV = [1.0] + [1.0 / (QL[j + 1] - QL[j]) for j in range(1, 11)]

# The device Ln table is only accurate for inputs in ~[1e-19, 1e19], but we
# need ln(w + 1e-30) with w in {0} u [3e-8, 1].  So compute
# Ln(w * 2^50 + 1e-30 * 2^50) on device (inputs then span [1.1e-15, 1.1e15])
# and subtract 50*ln2 on the host.
LN_SCALE = float(np.float32(2.0**50))
LN_BIAS = float(np.float32(np.float64(np.float32(1e-30)) * 2.0**50))
LN_OFFSET = np.float32(50.0 * np.log(np.float64(2.0)))

# channels whose affine+relu (y_j = relu(d*inv_j - 1)) runs on ACT; the
# rest compute z on DVE (GPSIMD is ~20x slower than DVE for fp32
# elementwise and throttles concurrent DVE via shared SBUF ports — avoid).
ACT_Z_CHANNELS = frozenset(range(1, 9))


def build_program():
    nc = bacc.Bacc("TRN2", target_bir_lowering=False, debug=False, num_devices=NCORES)
    # register activation-bias constants (only 0.0/1.0 are pre-registered)
    for name, val in (("lnbias", LN_BIAS), ("negone", -1.0)):
        ct = nc.alloc_sbuf_tensor(f"const-float32-{name}", [128, 1], F32)
        nc.gpsimd.memset(ct.ap(), val)
        nc.const_aps.aps[(F32, val)] = ct.ap()
    nc.all_engine_barrier()
    d_ext = nc.declare_dram_parameter("degrees", [P, COLS], F32, isOutput=False)
    out_ext = nc.declare_dram_parameter("out", [P, K, COLS], F32, isOutput=True)

    with tile.TileContext(nc) as tc:
        with (
            tc.tile_pool(name="dp", bufs=2) as dp,
            tc.tile_pool(name="cp", bufs=2) as cp,
            tc.tile_pool(name="sa", bufs=2) as sa,
            tc.tile_pool(name="sb", bufs=2) as sb,
        ):
            for t in range(NT):
                d = dp.tile([P, F], F32, tag="d")
                nc.sync.dma_start(out=d[:], in_=d_ext[:, t * F : (t + 1) * F])

                # staging halves: channels 0-5 and 6-11
                stg_a = sa.tile([P, 6 * F], F32, tag="stg_a")
                stg_b = sb.tile([P, 6 * F], F32, tag="stg_b")

                def stg_slice(j):
                    return (
                        stg_a[:, j * F : (j + 1) * F]
                        if j < 6
                        else stg_b[:, (j - 6) * F : (j - 5) * F]
                    )

                c = []
                for j in range(11):
                    cj = cp.tile([P, F], F32, tag=f"c{j}")
                    if j == 0:
                        # c_0 = clip(d, 0, 1)
                        nc.vector.tensor_scalar(cj[:], d[:], 0.0, 1.0, OP.max, OP.min)
                    elif j in ACT_Z_CHANNELS:
                        # y_j = relu(d*inv_j - 1) on ACT, then min(.,1) on DVE
                        nc.scalar.activation(
                            cj[:], d[:], AF.Relu, bias=-1.0, scale=INV[j]
                        )
                        nc.vector.tensor_scalar(cj[:], cj[:], 1.0, None, OP.min)
                    else:
                        # z_j = d*inv_j - 1, then clip, all on DVE
                        nc.vector.tensor_scalar(
                            cj[:], d[:], INV[j], 1.0, OP.mult, OP.subtract
                        )
                        nc.vector.tensor_scalar(cj[:], cj[:], 0.0, 1.0, OP.max, OP.min)
                    c.append(cj)

                # w_0 = 1 - c_0
                nc.vector.tensor_scalar(
                    stg_slice(0), c[0][:], -1.0, 1.0, OP.mult, OP.add
                )
                # w_j = c_{j-1} - c_j
                for j in range(1, 11):
                    nc.vector.tensor_tensor(
                        stg_slice(j), c[j - 1][:], c[j][:], OP.subtract
                    )

                # out = ln(w*2^50 + bias); channel 11 reads c_10 directly
                nc.scalar.activation(
                    stg_a[:, :], stg_a[:, :], AF.Ln, bias=LN_BIAS, scale=LN_SCALE
                )
                nc.scalar.activation(
                    stg_b[:, 0 : 5 * F],
                    stg_b[:, 0 : 5 * F],
                    AF.Ln,
                    bias=LN_BIAS,
                    scale=LN_SCALE,
                )
                nc.scalar.activation(
                    stg_b[:, 5 * F : 6 * F],
                    c[10][:],
                    AF.Ln,
                    bias=LN_BIAS,
                    scale=LN_SCALE,
                )

                nc.sync.dma_start(
                    out=out_ext[:, 0:6, t * F : (t + 1) * F],
                    in_=stg_a[:, :].rearrange("p (j f) -> p j f", j=6),
                )
                nc.sync.dma_start(
                    out=out_ext[:, 6:12, t * F : (t + 1) * F],
                    in_=stg_b[:, :].rearrange("p (j f) -> p j f", j=6),
                )
    nc.compile()
    return nc


_CACHE = {}
RUN_KWARGS = {}  # test harness can set e.g. {"trace": True} for profiling


def kernel(degrees, quantile_values):
    q = np.asarray(quantile_values, dtype=np.float32)
    assert np.array_equal(q, np.array(QL, dtype=np.float32)), "unexpected quantile grid"

    deg = np.ascontiguousarray(np.asarray(degrees, dtype=np.float32)[..., 0])  # (B,S)
    shards = deg.reshape(NCORES, P, COLS)

    if "nc" not in _CACHE:
        _CACHE["nc"] = build_program()
    nc = _CACHE["nc"]

    in_maps = [{"degrees": np.ascontiguousarray(shards[i])} for i in range(NCORES)]
    res = run_bass_kernel_spmd(nc, in_maps, list(range(NCORES)), **RUN_KWARGS)
    _CACHE["last_result"] = res
    outs = np.stack([res.results[i]["out"] for i in range(NCORES)])  # (8,128,12,2048)

    full = (
        outs.transpose(0, 1, 3, 2)  # (8,128,2048,12) — element order, channel last
        .reshape(B, S, K)
        .astype(np.float32, copy=True)
    )
    full -= LN_OFFSET
    full[deg >= np.float32(1024.0)] = np.float32(0.0)
    return full


# revision 14
# speedup vs baseline: 7.9675x; 1.1293x over previous
"""DegreeQuantileConverter Trainium2 kernel.

deg (B,S,1) f32 -> out (B,S,12) f32 = log(w + 1e-30) where w are the
piecewise-linear interpolation weights of deg onto the quantile grid
q = [0,1,2,4,...,1024], with rows where deg >= 1024 forced to w = 1.

Math: with c_j = clip((d - q_j)/(q_{j+1}-q_j), 0, 1) for j=0..10 the
weights telescope:  w_0 = 1-c_0, w_j = c_{j-1}-c_j, w_11 = c_10.
Since q_j/(q_{j+1}-q_j) == 1 for j>=1, z_j = d*inv_j - 1 (inv_j a power
of two), which keeps every value bit-identical to the reference's
(d-lo)/(hi-lo) path.  The deg>=1024 all-ones override is applied on the
host (cheap boolean mask on the gathered result).

Sharding: batch 128 -> 16 rows per core x 8 cores, each core sees its
shard as [128 partitions x 2048 cols]; output is written channel-major
[128, 12, 2048] per core and re-interleaved on the host.
"""

import numpy as np

import concourse.bacc as bacc
import concourse.mybir as mybir
import concourse.tile as tile
from concourse.bass_utils import run_bass_kernel_spmd

AF = mybir.ActivationFunctionType
OP = mybir.AluOpType
F32 = mybir.dt.float32

B, S, K = 128, 16384, 12
NCORES = 8
P = 128
ELEMS = (B // NCORES) * S      # 262144 per core
COLS = ELEMS // P              # 2048
F = 1024                       # free-dim tile size
NT = COLS // F                 # 2 tiles per core

QL = [0.0, 1.0, 2.0, 4.0, 8.0, 16.0, 32.0, 64.0, 128.0, 256.0, 512.0, 1024.0]
INV = [1.0] + [1.0 / (QL[j + 1] - QL[j]) for j in range(1, 11)]

# The device Ln table is only accurate for inputs in ~[1e-19, 1e19], but we
# need ln(w + 1e-30) with w in {0} u [3e-8, 1].  So compute
# Ln(w * 2^50 + 1e-30 * 2^50) on device (inputs then span [1.1e-15, 1.1e15])
# and subtract 50*ln2 on the host.
LN_SCALE = float(np.float32(2.0**50))
LN_BIAS = float(np.float32(np.float64(np.float32(1e-30)) * 2.0**50))
LN_OFFSET = np.float32(50.0 * np.log(np.float64(2.0)))

# channels whose affine+relu (y_j = relu(d*inv_j - 1)) runs on ACT; the
# rest compute z on DVE (GPSIMD is ~20x slower than DVE for fp32
# elementwise and throttles concurrent DVE via shared SBUF ports — avoid).
ACT_Z_CHANNELS = frozenset(range(1, 8))


def build_program():
    nc = bacc.Bacc("TRN2", target_bir_lowering=False, debug=False, num_devices=NCORES)
    # register activation-bias constants (only 0.0/1.0 are pre-registered)
    for name, val in (("lnbias", LN_BIAS), ("negone", -1.0)):
        ct = nc.alloc_sbuf_tensor(f"const-float32-{name}", [128, 1], F32)
        nc.gpsimd.memset(ct.ap(), val)
        nc.const_aps.aps[(F32, val)] = ct.ap()
    nc.all_engine_barrier()
    d_ext = nc.declare_dram_parameter("degrees", [P, COLS], F32, isOutput=False)
    out_ext = nc.declare_dram_parameter("out", [P, K, COLS], F32, isOutput=True)

    with tile.TileContext(nc) as tc:
        with (
            tc.tile_pool(name="dp", bufs=2) as dp,
            tc.tile_pool(name="cp", bufs=2) as cp,
            tc.tile_pool(name="sa", bufs=2) as sa,
            tc.tile_pool(name="sb", bufs=2) as sb,
        ):
            for t in range(NT):
                d = dp.tile([P, F], F32, tag="d")
                nc.sync.dma_start(out=d[:], in_=d_ext[:, t * F : (t + 1) * F])

                # staging halves: channels 0-5 and 6-11
                stg_a = sa.tile([P, 6 * F], F32, tag="stg_a")
                stg_b = sb.tile([P, 6 * F], F32, tag="stg_b")

                def stg_slice(j):
                    return (
                        stg_a[:, j * F : (j + 1) * F]
                        if j < 6
                        else stg_b[:, (j - 6) * F : (j - 5) * F]
                    )

                c = []
                for j in range(11):
                    cj = cp.tile([P, F], F32, tag=f"c{j}")
                    if j == 0:
                        # c_0 = clip(d, 0, 1)
                        nc.vector.tensor_scalar(cj[:], d[:], 0.0, 1.0, OP.max, OP.min)
                    elif j in ACT_Z_CHANNELS:
                        # y_j = relu(d*inv_j - 1) on ACT, then min(.,1) on DVE
                        nc.scalar.activation(
                            cj[:], d[:], AF.Relu, bias=-1.0, scale=INV[j]
                        )
                        nc.vector.tensor_scalar(cj[:], cj[:], 1.0, None, OP.min)
                    else:
                        # z_j = d*inv_j - 1, then clip, all on DVE
                        nc.vector.tensor_scalar(
                            cj[:], d[:], INV[j], 1.0, OP.mult, OP.subtract
                        )
                        nc.vector.tensor_scalar(cj[:], cj[:], 0.0, 1.0, OP.max, OP.min)
                    c.append(cj)

                # w_0 = 1 - c_0
                nc.vector.tensor_scalar(
                    stg_slice(0), c[0][:], -1.0, 1.0, OP.mult, OP.add
                )
                # w_j = c_{j-1} - c_j
                for j in range(1, 11):
                    nc.vector.tensor_tensor(
                        stg_slice(j), c[j - 1][:], c[j][:], OP.subtract
                    )

                # out = ln(w*2^50 + bias); channel 11 reads c_10 directly.
                # 2-channel Ln groups + 2-channel output DMAs so the store
                # stream starts as soon as the first diffs are done.
                def stg_pair(j):  # [128, 2F] slice holding channels j, j+1
                    return (
                        stg_a[:, j * F : (j + 2) * F]
                        if j < 6
                        else stg_b[:, (j - 6) * F : (j - 4) * F]
                    )

                for j in range(0, 10, 2):
                    nc.scalar.activation(
                        stg_pair(j), stg_pair(j), AF.Ln, bias=LN_BIAS, scale=LN_SCALE
                    )
                nc.scalar.activation(
                    stg_b[:, 4 * F : 5 * F],
                    stg_b[:, 4 * F : 5 * F],
                    AF.Ln,
                    bias=LN_BIAS,
                    scale=LN_SCALE,
                )
                nc.scalar.activation(
                    stg_b[:, 5 * F : 6 * F],
                    c[10][:],
                    AF.Ln,
                    bias=LN_BIAS,
                    scale=LN_SCALE,
                )

                for j in range(0, 12, 2):
                    nc.sync.dma_start(
                        out=out_ext[:, j : j + 2, t * F : (t + 1) * F],
                        in_=stg_pair(j).rearrange("p (j f) -> p j f", j=2),
                    )
    nc.compile()
    return nc


_CACHE = {}
RUN_KWARGS = {}  # test harness can set e.g. {"trace": True} for profiling


def kernel(degrees, quantile_values):
    q = np.asarray(quantile_values, dtype=np.float32)
    assert np.array_equal(q, np.array(QL, dtype=np.float32)), "unexpected quantile grid"

    deg = np.ascontiguousarray(np.asarray(degrees, dtype=np.float32)[..., 0])  # (B,S)
    shards = deg.reshape(NCORES, P, COLS)

    if "nc" not in _CACHE:
        _CACHE["nc"] = build_program()
    nc = _CACHE["nc"]

    in_maps = [{"degrees": np.ascontiguousarray(shards[i])} for i in range(NCORES)]
    res = run_bass_kernel_spmd(nc, in_maps, list(range(NCORES)), **RUN_KWARGS)
    _CACHE["last_result"] = res
    outs = np.stack([res.results[i]["out"] for i in range(NCORES)])  # (8,128,12,2048)

    full = (
        outs.transpose(0, 1, 3, 2)  # (8,128,2048,12) — element order, channel last
        .reshape(B, S, K)
        .astype(np.float32, copy=True)
    )
    full -= LN_OFFSET
    full[deg >= np.float32(1024.0)] = np.float32(0.0)
    return full
